# revision 3
# baseline (speedup 1.0000x reference)
"""Trainium2 Bass kernel for windowed Conv1d(k=3) + sigmoid gating.

Reference computation (B=16, T=960, D=1024, W=10):
  windows of size 10 are conv'd independently with per-window zero pad 1:
    cnn[t, d] = sum_{k,c} conv_w[d, c, k] * xpad[t + k, c] + conv_b[d]
    out = cnn * sigmoid(cnn @ gate_w.T + gate_b)

Strategy: pure data parallelism over the 8 NeuronCores (2 batches per
core, 192 windows = 1920 rows each), with the conv done as Winograd
F(2,3) in bf16 (see _build docstring below) and the gate matmul split
by column group between fp8e4m3 DoubleRow (2x PE throughput; groups
0-1) and bf16 (groups 2-3).  The fp8 half of the gate costs ~1.4e-2
rel err against the 2e-2 budget; conv stays bf16 (fp8 there measures
~6e-2, far over budget).

fp8 DoubleRow mechanics: the PE reconfigures to a 256-deep x 64-wide
array; lhsT is [128, 2, 64], rhs is [128, 2, N<=256], out is [64, N]
and MUST sit at PSUM partition 0 (the ISA rejects partition-64 bases
in DR mode).  The e-dimension is therefore processed in 64-row chunks
whose results land on partitions 0:64; the "hi" chunk of each 128-row
e-block is realigned to partitions 64:128 with a small SBUF->SBUF DMA
after the sigmoid so the final multiply + store run at full width.

Schedule: convs for groups 0,1 first (producing bf16 cnn tiles plus an
fp8 copy); then convs for groups 2,3 with the fp8 gates of groups 0,1
interleaved between conv steps (they hide under the conv stream); the
bf16 gates of groups 2,3 form the tail, where their bigger matmuls
balance the sigmoid/mul/store epilogue chain.
"""

import numpy as np
import ml_dtypes

import concourse.bacc as bacc
import concourse.bass as bass
import concourse.tile as tile
from concourse import mybir
from concourse.bass_utils import run_bass_kernel_spmd

BF16 = ml_dtypes.bfloat16
F8 = ml_dtypes.float8_e4m3

B, T, D, W = 16, 960, 1024, 10
NCORES = 8
BC = B // NCORES            # batches per core
NWIN = BC * T // W          # windows per core (192)
RC = NWIN * W               # output rows per core (1920)
PW = W + 2                  # padded window length (12)
NP = W // 2                 # winograd output pairs per window (5)
NG = 4                      # column groups per core
NF8 = 2                     # groups whose gate runs in fp8 DoubleRow
GWIN = NWIN // NG           # windows per group (48)
GN = GWIN * W               # output columns per group (480)
GM = NP * GWIN              # winograd columns per group (240)
NCH = D // 128              # 128-partition chunks of D (8)
NE64 = D // 64              # 64-row chunks of the gate output dim (16)
AF = mybir.ActivationFunctionType
DR = mybir.MatmulPerfMode.DoubleRow


def _build():
    nc = bacc.Bacc("TRN2", target_bir_lowering=False, debug=False)

    # xt: [group, cc, (ck, j, pair, win)] winograd-transformed input, one
    # contiguous block per group so each group loads with a single DMA
    xt = nc.dram_tensor("xt", [NG, 128, NCH * 4 * GM], mybir.dt.bfloat16,
                        kind="ExternalInput")
    # cwr[dck]: [cc, ((j*NCH+ck)*128 + dd)] winograd conv lhsT blocks
    cwr = nc.dram_tensor("cwr", [NCH, 128, 4 * NCH * 128], mybir.dt.bfloat16,
                         kind="ExternalInput")
    # gwr[eck]: [dd, (dck*128 + ee)] gate lhsT blocks (bf16 groups)
    gwr = nc.dram_tensor("gwr", [NCH, 128, NCH * 128], mybir.dt.bfloat16,
                         kind="ExternalInput")
    # gw8: [dd, ((m*4+s)*128 + i*64 + ee)] fp8 DoubleRow gate lhsT blocks:
    # block (m, s) holds gate_w[e=m*64+ee, d=(s*2+i)*128+dd]
    gw8 = nc.dram_tensor("gw8", [128, NE64 * 4 * 128], mybir.dt.float8e4,
                         kind="ExternalInput")
    cb = nc.dram_tensor("cb", [128, NCH], mybir.dt.float32, kind="ExternalInput")
    gb = nc.dram_tensor("gb", [128, NCH], mybir.dt.float32, kind="ExternalInput")
    # gb64[ee, m] = gate_b[m*64 + ee] - bias aligned to partition 0:64
    gb64 = nc.dram_tensor("gb64", [64, NE64], mybir.dt.float32,
                          kind="ExternalInput")
    outT = nc.dram_tensor("outT", [D, RC], mybir.dt.bfloat16,
                          kind="ExternalOutput")

    with tile.TileContext(nc) as tc:
        with (
            tc.tile_pool(name="consts", bufs=1) as consts,
            tc.tile_pool(name="work", bufs=2) as work,
            tc.tile_pool(name="cnn", bufs=4) as cnnp,
            tc.tile_pool(name="cpsum", bufs=3, space="PSUM") as cpsum,
            tc.tile_pool(name="gpsum", bufs=2, space="PSUM") as gpsum,
        ):
            xt_sb = [None] * NG

            def load_xg(g):
                t = consts.tile([128, NCH * 4 * GM], mybir.dt.bfloat16, tag=f"xg{g}")
                nc.sync.dma_start(t[:], xt[g])
                xt_sb[g] = t

            cwr_sb = [None] * NCH

            def load_cw(dck):
                t = consts.tile([128, 4 * NCH * 128], mybir.dt.bfloat16,
                                tag=f"cw{dck}")
                nc.sync.dma_start(t[:], cwr[dck])
                cwr_sb[dck] = t

            # The per-core HBM read bandwidth (~350 GB/s) is shared by both
            # HWDGE queues, so all inputs go on one queue (Sync) in exact
            # first-use order; ScalarE stays free for the epilogue copies.
            load_xg(0)
            load_cw(0)
            cb_sb = consts.tile([128, NCH], mybir.dt.float32, tag="cb")
            nc.sync.dma_start(cb_sb[:], cb[:])
            gb_sb = consts.tile([128, NCH], mybir.dt.float32, tag="gb")
            nc.sync.dma_start(gb_sb[:], gb[:])
            gb64_sb = consts.tile([64, NE64], mybir.dt.float32, tag="gb64")
            nc.sync.dma_start(gb64_sb[:], gb64[:])
            load_xg(1)
            load_cw(1)
            load_cw(2)
            load_cw(3)
            load_xg(2)
            load_cw(4)
            load_cw(5)
            load_xg(3)
            load_cw(6)
            load_cw(7)

            # fp8 gate weights (needed right after pair-A convs)
            gw8_sb = consts.tile([128, NE64 * 4 * 128], mybir.dt.float8e4,
                                 tag="gw8")
            nc.sync.dma_start(gw8_sb[:], gw8[:])

            gwr_sb = []
            for eck in range(NCH):
                t = consts.tile([128, NCH * 128], mybir.dt.bfloat16, tag=f"gw{eck}")
                nc.sync.dma_start(t[:], gwr[eck])
                gwr_sb.append(t)

            # fp8 copies of cnn for the DoubleRow gate groups
            cnn8_sb = [
                consts.tile([128, NCH * GN], mybir.dt.float8e4, tag=f"cnn8_{g}",
                            name=f"cnn8_{g}")
                for g in range(NF8)
            ]

            # Warm-up during the input-DMA bubble: a burst of throwaway
            # matmuls flips the PE HAM clock gate to 8/8 just as the real
            # stream starts.
            scr = consts.tile([128, 512], mybir.dt.bfloat16, tag="scr")
            nc.gpsimd.memset(scr[:], 0.0)
            wps = gpsum.tile([128, GN], mybir.dt.float32, tag="gps")
            for _ in range(16):
                nc.tensor.matmul(wps[:, :480], scr[:, :128], scr[:, :480],
                                 start=True, stop=True)

            def conv_tile(g, dck, make_fp8):
                ps = cpsum.tile([128, 4, 256], mybir.dt.float32, tag="cps")
                # j outer: a start=True clears has_written for its whole
                # PSUM bank, so the two j-groups sharing a bank must not
                # interleave their accumulation.
                for j in range(4):
                    for ck in range(NCH):
                        nc.tensor.matmul(
                            ps[:, j, :GM],
                            cwr_sb[dck][:, (j * NCH + ck) * 128:
                                        (j * NCH + ck + 1) * 128],
                            xt_sb[g][:, (ck * 4 + j) * GM:(ck * 4 + j + 1) * GM],
                            start=(ck == 0),
                            stop=(ck == NCH - 1),
                        )
                # A^T combine: y0 = m1+m2+m3+cb, y1 = m2-m3-m4+cb
                cbs = cb_sb[:, dck:dck + 1]
                m2s = work.tile([128, GM], mybir.dt.bfloat16, tag="m2s")
                nc.scalar.activation(m2s[:], ps[:, 1, :GM], AF.Copy)
                m3s = work.tile([128, GM], mybir.dt.bfloat16, tag="m3s")
                nc.scalar.activation(m3s[:], ps[:, 2, :GM], AF.Copy)
                t0 = work.tile([128, GM], mybir.dt.bfloat16, tag="t0")
                nc.vector.tensor_scalar_add(t0[:], ps[:, 0, :GM], cbs)
                u = work.tile([128, GM], mybir.dt.bfloat16, tag="u")
                nc.vector.tensor_scalar(u[:], ps[:, 3, :GM], cbs, None,
                                        mybir.AluOpType.subtract)
                ct = cnnp.tile([128, GN], mybir.dt.bfloat16, tag=f"cnn{dck}")
                ctv = ct[:].rearrange("q (p two w) -> q two p w", two=2, w=GWIN)
                a = work.tile([128, GM], mybir.dt.bfloat16, tag="a")
                nc.vector.tensor_add(a[:], t0[:], m2s[:])
                nc.vector.tensor_add(ctv[:, 0], a[:], m3s[:])
                v = work.tile([128, GM], mybir.dt.bfloat16, tag="v")
                nc.vector.tensor_sub(v[:], m2s[:], m3s[:])
                nc.vector.tensor_sub(ctv[:, 1], v[:], u[:])
                if make_fp8:
                    # fp8 copy for the DoubleRow gate (rhs layout [dck, col])
                    nc.scalar.activation(
                        cnn8_sb[g][:, dck * GN:(dck + 1) * GN], ct[:], AF.Copy)
                return ct

            def gate_eck_fp8(g, eck, cnnT):
                # gate for e-rows [eck*128, (eck+1)*128) of fp8 group g, via
                # two 64-row DoubleRow chunks computed at PSUM partitions 0:64
                cn8 = cnn8_sb[g][:].rearrange("p (d n) -> p d n", d=NCH)
                gt = work.tile([128, GN], mybir.dt.bfloat16, tag="gate")
                for half in (1, 0):       # hi first: its realign DMA overlaps
                    m = eck * 2 + half
                    ps8 = gpsum.tile([128, GN], mybir.dt.float32, tag="gps")
                    for lo, hi in ((0, GN // 2), (GN // 2, GN)):
                        for s in range(4):
                            wv = gw8_sb[:, (m * 4 + s) * 128:
                                        (m * 4 + s + 1) * 128]
                            nc.tensor.matmul(
                                ps8[0:64, lo:hi],
                                wv.rearrange("p (i e) -> p i e", i=2),
                                cn8[:, 2 * s:2 * s + 2, lo:hi],
                                start=(s == 0),
                                stop=(s == 3),
                                perf_mode=DR,
                            )
                    if half:
                        ghi = work.tile([64, GN], mybir.dt.bfloat16, tag="ghi")
                        nc.scalar.activation(ghi[:], ps8[0:64, :], AF.Sigmoid,
                                             bias=gb64_sb[:, m:m + 1])
                        # realign e-rows 64:128 onto partitions 64:128
                        nc.sync.dma_start(gt[64:128, :], ghi[:])
                    else:
                        nc.scalar.activation(gt[0:64, :], ps8[0:64, :],
                                             AF.Sigmoid,
                                             bias=gb64_sb[:, m:m + 1])
                ot = work.tile([128, GN], mybir.dt.bfloat16, tag="out")
                nc.vector.tensor_mul(ot[:], cnnT[eck][:], gt[:])
                nc.sync.dma_start(
                    outT[eck * 128:(eck + 1) * 128, g * GN:(g + 1) * GN], ot[:])

            def gate_group(g, cnnT, last=False):
                # gate: gateT[e, r] = sigmoid(sum_d gw[d, e] * cnnT[d, r] + gb[e])
                for eck in range(NCH):
                    ps2 = gpsum.tile([128, GN], mybir.dt.float32, tag="gps")
                    for dck in range(NCH):
                        nc.tensor.matmul(
                            ps2[:],
                            gwr_sb[eck][:, dck * 128:(dck + 1) * 128],
                            cnnT[dck][:],
                            start=(dck == 0),
                            stop=(dck == NCH - 1),
                        )
                    gt = work.tile([128, GN], mybir.dt.bfloat16, tag="gate")
                    ot = work.tile([128, GN], mybir.dt.bfloat16, tag="out")
                    # the last tile's sigmoid->mul->DMA chain is the kernel
                    # tail: run it in half-tiles so the stages overlap
                    chunks = ((0, GN // 2), (GN // 2, GN)) if (
                        last and eck == NCH - 1) else ((0, GN),)
                    for lo, hi in chunks:
                        nc.scalar.activation(gt[:, lo:hi], ps2[:, lo:hi],
                                             AF.Sigmoid,
                                             bias=gb_sb[:, eck:eck + 1])
                        nc.vector.tensor_mul(ot[:, lo:hi], cnnT[eck][:, lo:hi],
                                             gt[:, lo:hi])
                        nc.sync.dma_start(
                            outT[eck * 128:(eck + 1) * 128,
                                 g * GN + lo:g * GN + hi], ot[:, lo:hi]
                        )

            cnn_tiles = [[None] * NCH for _ in range(NG)]
            # pair A convs (fp8-gated groups)
            for dck in range(NCH):
                for g in range(NF8):
                    cnn_tiles[g][dck] = conv_tile(g, dck, make_fp8=True)
            # pair B convs with pair A's fp8 gates interleaved beneath them
            for dck in range(NCH):
                for g in range(NF8, NG):
                    cnn_tiles[g][dck] = conv_tile(g, dck, make_fp8=False)
                for g in range(NF8):
                    gate_eck_fp8(g, dck, cnn_tiles[g])
            # bf16 gates form the tail
            for g in range(NF8, NG):
                gate_group(g, cnn_tiles[g], last=(g == NG - 1))
    nc.compile()
    return nc


def _prep_core_input(x_shard, cw_host, gw_host, gw8_host, cb_host, gb_host,
                     gb64_host):
    # x_shard: [BC, T, D] -> padded transposed [D, NG, PW, GWIN]
    xs = x_shard.reshape(NG, GWIN, W, D)
    xp = np.zeros((D, NG, PW, GWIN), np.float32)
    xp[:, :, 1:1 + W, :] = xs.transpose(3, 0, 2, 1)
    # winograd input transform, pairs p: x_i = xp[2p + i]
    x0 = xp[:, :, 0:2 * NP:2]
    x1 = xp[:, :, 1:1 + 2 * NP:2]
    x2 = xp[:, :, 2:2 + 2 * NP:2]
    x3 = xp[:, :, 3:3 + 2 * NP:2]
    xt = np.stack([x0 - x2, x1 + x2, x2 - x1, x1 - x3], axis=2)  # [D,NG,4,NP,GWIN]
    # -> [g, cc, (ck, j, p, w)] so each group is one contiguous DMA
    xt = xt.reshape(NCH, 128, NG, 4, NP, GWIN).transpose(2, 1, 0, 3, 4, 5)
    xt_host = np.ascontiguousarray(xt).astype(BF16).reshape(NG, 128, NCH * 4 * GM)
    return {"xt": xt_host, "cwr": cw_host, "gwr": gw_host, "gw8": gw8_host,
            "cb": cb_host, "gb": gb_host, "gb64": gb64_host}


def _prep_in_maps(x, conv_w, conv_b, gate_w, gate_b):
    # winograd weight transform + lhsT blocks:
    # cwr[dck][cc, (j*NCH+ck)*128 + dd] = Wt_j[dck*128+dd, ck*128+cc]
    W0, W1, W2 = conv_w[:, :, 0], conv_w[:, :, 1], conv_w[:, :, 2]
    wt = np.stack([W0, (W0 + W1 + W2) * 0.5, (W0 - W1 + W2) * 0.5, W2])  # [4,d,c]
    wt = wt.reshape(4, NCH, 128, NCH, 128)  # [j, dck, dd, ck, cc]
    cw_host = np.ascontiguousarray(wt.transpose(1, 4, 0, 3, 2)).reshape(
        NCH, 128, 4 * NCH * 128).astype(BF16)
    # gate lhsT blocks: gwr[eck][dd, dck*128 + ee] = gate_w[eck*128+ee, dck*128+dd]
    gwt = gate_w.T.reshape(NCH, 128, NCH, 128)  # [dck, dd, eck, ee]
    gw_host = np.ascontiguousarray(gwt.transpose(2, 1, 0, 3)).reshape(
        NCH, 128, NCH * 128).astype(BF16)
    # fp8 DoubleRow gate blocks: [dd, (m, s, i, ee)]
    g8 = gate_w.reshape(NE64, 64, 4, 2, 128)  # [m, ee, s, i, dd]
    gw8_host = np.ascontiguousarray(g8.transpose(4, 0, 2, 3, 1)).reshape(
        128, NE64 * 4 * 128).astype(F8)
    cb_host = np.ascontiguousarray(conv_b.reshape(NCH, 128).T).astype(np.float32)
    gb_host = np.ascontiguousarray(gate_b.reshape(NCH, 128).T).astype(np.float32)
    gb64_host = np.ascontiguousarray(gate_b.reshape(NE64, 64).T).astype(np.float32)
    return [
        _prep_core_input(x[BC * i:BC * (i + 1)], cw_host, gw_host, gw8_host,
                         cb_host, gb_host, gb64_host)
        for i in range(NCORES)
    ]


def _unshard_core(o):
    # o: [D, RC] with columns ordered (group, t, win) -> [BC, T, D]
    return (np.asarray(o).astype(np.float32)
            .reshape(D, NG, W, GWIN).transpose(1, 3, 2, 0)
            .reshape(NWIN, W, D).reshape(BC, T, D))


_NC_CACHE = None


def kernel(x, conv_w, conv_b, gate_w, gate_b):
    global _NC_CACHE
    x = np.asarray(x, np.float32)
    conv_w = np.asarray(conv_w, np.float32)
    conv_b = np.asarray(conv_b, np.float32)
    gate_w = np.asarray(gate_w, np.float32)
    gate_b = np.asarray(gate_b, np.float32)

    in_maps = _prep_in_maps(x, conv_w, conv_b, gate_w, gate_b)
    if _NC_CACHE is None:
        _NC_CACHE = _build()
    res = run_bass_kernel_spmd(_NC_CACHE, in_maps, core_ids=list(range(NCORES))).results

    out = np.empty((B, T, D), np.float32)
    for i in range(NCORES):
        out[BC * i:BC * (i + 1)] = _unshard_core(res[i]["outT"])
    return out


# revision 7
# speedup vs baseline: 1.0005x; 1.0005x over previous
"""Trainium2 Bass kernel for windowed Conv1d(k=3) + sigmoid gating.

Reference computation (B=16, T=960, D=1024, W=10):
  windows of size 10 are conv'd independently with per-window zero pad 1:
    cnn[t, d] = sum_{k,c} conv_w[d, c, k] * xpad[t + k, c] + conv_b[d]
    out = cnn * sigmoid(cnn @ gate_w.T + gate_b)

Strategy: pure data parallelism over the 8 NeuronCores (2 batches per
core, 192 windows = 1920 rows each), with the conv done as Winograd
F(2,3) in bf16 (see _build docstring below) and the gate matmul split
by column group between fp8e4m3 DoubleRow (2x PE throughput; groups
0-1) and bf16 (groups 2-3).  The fp8 half of the gate costs ~1.4e-2
rel err against the 2e-2 budget; conv stays bf16 (fp8 there measures
~6e-2, far over budget).

fp8 DoubleRow mechanics: the PE reconfigures to a 256-deep x 64-wide
array; lhsT is [128, 2, 64], rhs is [128, 2, N<=256], out is [64, N]
and MUST sit at PSUM partition 0 (the ISA rejects partition-64 bases
in DR mode).  The e-dimension is therefore processed in 64-row chunks
whose results land on partitions 0:64; the "hi" chunk of each 128-row
e-block is realigned to partitions 64:128 with a small SBUF->SBUF DMA
after the sigmoid so the final multiply + store run at full width.

Schedule: convs for groups 0,1 first (producing bf16 cnn tiles plus an
fp8 copy); then convs for groups 2,3 with the fp8 gates of groups 0,1
interleaved between conv steps (they hide under the conv stream); the
bf16 gates of groups 2,3 form the tail, where their bigger matmuls
balance the sigmoid/mul/store epilogue chain.
"""

import numpy as np
import ml_dtypes

import concourse.bacc as bacc
import concourse.bass as bass
import concourse.tile as tile
from concourse import mybir
from concourse.bass_utils import run_bass_kernel_spmd

BF16 = ml_dtypes.bfloat16
F8 = ml_dtypes.float8_e4m3

B, T, D, W = 16, 960, 1024, 10
NCORES = 8
BC = B // NCORES            # batches per core
NWIN = BC * T // W          # windows per core (192)
RC = NWIN * W               # output rows per core (1920)
PW = W + 2                  # padded window length (12)
NP = W // 2                 # winograd output pairs per window (5)
NG = 4                      # column groups per core
NF8 = 2                     # groups whose gate runs in fp8 DoubleRow
GWIN = NWIN // NG           # windows per group (48)
GN = GWIN * W               # output columns per group (480)
GM = NP * GWIN              # winograd columns per group (240)
NCH = D // 128              # 128-partition chunks of D (8)
NE64 = D // 64              # 64-row chunks of the gate output dim (16)
AF = mybir.ActivationFunctionType
DR = mybir.MatmulPerfMode.DoubleRow


def _build():
    nc = bacc.Bacc("TRN2", target_bir_lowering=False, debug=False)

    # xt: [group, cc, (ck, j, pair, win)] winograd-transformed input, one
    # contiguous block per group so each group loads with a single DMA
    xt = nc.dram_tensor("xt", [NG, 128, NCH * 4 * GM], mybir.dt.bfloat16,
                        kind="ExternalInput")
    # cwr[dck]: [cc, ((j*NCH+ck)*128 + dd)] winograd conv lhsT blocks
    cwr = nc.dram_tensor("cwr", [NCH, 128, 4 * NCH * 128], mybir.dt.bfloat16,
                         kind="ExternalInput")
    # gwr[eck]: [dd, (dck*128 + ee)] gate lhsT blocks (bf16 groups)
    gwr = nc.dram_tensor("gwr", [NCH, 128, NCH * 128], mybir.dt.bfloat16,
                         kind="ExternalInput")
    # gw8: [dd, ((m*4+s)*128 + i*64 + ee)] fp8 DoubleRow gate lhsT blocks:
    # block (m, s) holds gate_w[e=m*64+ee, d=(s*2+i)*128+dd]
    gw8 = nc.dram_tensor("gw8", [128, NE64 * 4 * 128], mybir.dt.float8e4,
                         kind="ExternalInput")
    cb = nc.dram_tensor("cb", [128, NCH], mybir.dt.float32, kind="ExternalInput")
    gb = nc.dram_tensor("gb", [128, NCH], mybir.dt.float32, kind="ExternalInput")
    # gb64[ee, m] = gate_b[m*64 + ee] - bias aligned to partition 0:64
    gb64 = nc.dram_tensor("gb64", [64, NE64], mybir.dt.float32,
                          kind="ExternalInput")
    outT = nc.dram_tensor("outT", [D, RC], mybir.dt.bfloat16,
                          kind="ExternalOutput")

    with tile.TileContext(nc) as tc:
        with (
            tc.tile_pool(name="consts", bufs=1) as consts,
            tc.tile_pool(name="work", bufs=2) as work,
            tc.tile_pool(name="cnn", bufs=4) as cnnp,
            tc.tile_pool(name="cpsum", bufs=2, space="PSUM") as cpsum,
            tc.tile_pool(name="gpsum", bufs=1, space="PSUM") as gpsum,
        ):
            xt_sb = [None] * NG

            def load_xg(g):
                t = consts.tile([128, NCH * 4 * GM], mybir.dt.bfloat16, tag=f"xg{g}")
                nc.sync.dma_start(t[:], xt[g])
                xt_sb[g] = t

            cwr_sb = [None] * NCH

            def load_cw(dck):
                t = consts.tile([128, 4 * NCH * 128], mybir.dt.bfloat16,
                                tag=f"cw{dck}")
                nc.sync.dma_start(t[:], cwr[dck])
                cwr_sb[dck] = t

            # The per-core HBM read bandwidth (~350 GB/s) is shared by both
            # HWDGE queues, so all inputs go on one queue (Sync) in exact
            # first-use order; ScalarE stays free for the epilogue copies.
            load_xg(0)
            load_cw(0)
            cb_sb = consts.tile([128, NCH], mybir.dt.float32, tag="cb")
            nc.sync.dma_start(cb_sb[:], cb[:])
            gb_sb = consts.tile([128, NCH], mybir.dt.float32, tag="gb")
            nc.sync.dma_start(gb_sb[:], gb[:])
            gb64_sb = consts.tile([64, NE64], mybir.dt.float32, tag="gb64")
            nc.sync.dma_start(gb64_sb[:], gb64[:])
            load_xg(1)
            load_cw(1)
            load_cw(2)
            load_cw(3)
            load_xg(2)
            load_cw(4)
            load_cw(5)
            load_xg(3)
            load_cw(6)
            load_cw(7)

            # fp8 gate weights (needed right after pair-A convs)
            gw8_sb = consts.tile([128, NE64 * 4 * 128], mybir.dt.float8e4,
                                 tag="gw8")
            nc.sync.dma_start(gw8_sb[:], gw8[:])

            gwr_sb = []
            for eck in range(NCH):
                t = consts.tile([128, NCH * 128], mybir.dt.bfloat16, tag=f"gw{eck}")
                nc.sync.dma_start(t[:], gwr[eck])
                gwr_sb.append(t)

            # fp8 copies of cnn for the DoubleRow gate groups
            cnn8_sb = [
                consts.tile([128, NCH * GN], mybir.dt.float8e4, tag=f"cnn8_{g}",
                            name=f"cnn8_{g}")
                for g in range(NF8)
            ]

            # Warm-up during the input-DMA bubble: a burst of throwaway
            # matmuls flips the PE HAM clock gate to 8/8 just as the real
            # stream starts.
            scr = consts.tile([128, 512], mybir.dt.bfloat16, tag="scr")
            nc.gpsimd.memset(scr[:], 0.0)
            wps = gpsum.tile([128, GN], mybir.dt.float32, tag="gpsA")
            for _ in range(28):
                nc.tensor.matmul(wps[:, :480], scr[:, :128], scr[:, :480],
                                 start=True, stop=True)

            def conv_tile(g, dck, make_fp8):
                ps = cpsum.tile([128, 4, 256], mybir.dt.float32, tag="cps")
                # j outer: a start=True clears has_written for its whole
                # PSUM bank, so the two j-groups sharing a bank must not
                # interleave their accumulation.
                for j in range(4):
                    for ck in range(NCH):
                        nc.tensor.matmul(
                            ps[:, j, :GM],
                            cwr_sb[dck][:, (j * NCH + ck) * 128:
                                        (j * NCH + ck + 1) * 128],
                            xt_sb[g][:, (ck * 4 + j) * GM:(ck * 4 + j + 1) * GM],
                            start=(ck == 0),
                            stop=(ck == NCH - 1),
                        )
                # A^T combine: y0 = m1+m2+m3+cb, y1 = m2-m3-m4+cb
                cbs = cb_sb[:, dck:dck + 1]
                m2s = work.tile([128, GM], mybir.dt.bfloat16, tag="m2s")
                nc.scalar.activation(m2s[:], ps[:, 1, :GM], AF.Copy)
                m3s = work.tile([128, GM], mybir.dt.bfloat16, tag="m3s")
                nc.scalar.activation(m3s[:], ps[:, 2, :GM], AF.Copy)
                t0 = work.tile([128, GM], mybir.dt.bfloat16, tag="t0")
                nc.vector.tensor_scalar_add(t0[:], ps[:, 0, :GM], cbs)
                u = work.tile([128, GM], mybir.dt.bfloat16, tag="u")
                nc.vector.tensor_scalar(u[:], ps[:, 3, :GM], cbs, None,
                                        mybir.AluOpType.subtract)
                ct = cnnp.tile([128, GN], mybir.dt.bfloat16, tag=f"cnn{dck}")
                ctv = ct[:].rearrange("q (p two w) -> q two p w", two=2, w=GWIN)
                a = work.tile([128, GM], mybir.dt.bfloat16, tag="a")
                nc.vector.tensor_add(a[:], t0[:], m2s[:])
                nc.vector.tensor_add(ctv[:, 0], a[:], m3s[:])
                v = work.tile([128, GM], mybir.dt.bfloat16, tag="v")
                nc.vector.tensor_sub(v[:], m2s[:], m3s[:])
                nc.vector.tensor_sub(ctv[:, 1], v[:], u[:])
                if make_fp8:
                    # fp8 copy for the DoubleRow gate (rhs layout [dck, col])
                    nc.scalar.activation(
                        cnn8_sb[g][:, dck * GN:(dck + 1) * GN], ct[:], AF.Copy)
                return ct

            HGN = GN // 2

            def gate_eck_fp8(eck, cnn_tiles, gts):
                # gate rows [eck*128, (eck+1)*128) for BOTH fp8 groups at
                # once: one DoubleRow weight load feeds 4 matmuls (2 groups
                # x 2 column chunks), so the ~97ns LDWEIGHTS hides under
                # ~200ns of preceding stream instead of a 50ns DR matmul.
                # The 4 open accumulations live in 4 separate PSUM banks.
                for half in (1, 0):       # hi first: its realign DMA overlaps
                    m = eck * 2 + half
                    ps8 = [gpsum.tile([64, HGN], mybir.dt.float32,
                                      tag=f"gps{t}", name=f"ps8{t}")
                           for t in "ABCD"]
                    for s in range(4):
                        wv = gw8_sb[:, (m * 4 + s) * 128:
                                    (m * 4 + s + 1) * 128]
                        wv = wv.rearrange("p (i e) -> p i e", i=2)
                        for g in range(NF8):
                            cn8 = cnn8_sb[g][:].rearrange(
                                "p (d n) -> p d n", d=NCH)
                            for c, (lo, hi) in enumerate(((0, HGN), (HGN, GN))):
                                nc.tensor.matmul(
                                    ps8[g * 2 + c][:],
                                    wv,
                                    cn8[:, 2 * s:2 * s + 2, lo:hi],
                                    start=(s == 0),
                                    stop=(s == 3),
                                    perf_mode=DR,
                                )
                    for g in range(NF8):
                        if half:
                            ghi = work.tile([64, GN], mybir.dt.bfloat16,
                                            tag="ghi")
                            for c, lo in ((0, 0), (1, HGN)):
                                nc.scalar.activation(
                                    ghi[:, lo:lo + HGN], ps8[g * 2 + c][:],
                                    AF.Sigmoid, bias=gb64_sb[:, m:m + 1])
                            # realign e-rows 64:128 onto partitions 64:128
                            nc.sync.dma_start(gts[g][64:128, :], ghi[:])
                        else:
                            for c, lo in ((0, 0), (1, HGN)):
                                nc.scalar.activation(
                                    gts[g][0:64, lo:lo + HGN], ps8[g * 2 + c][:],
                                    AF.Sigmoid, bias=gb64_sb[:, m:m + 1])
                for g in range(NF8):
                    ot = work.tile([128, GN], mybir.dt.bfloat16, tag="out")
                    nc.vector.tensor_mul(ot[:], cnn_tiles[g][eck][:], gts[g][:])
                    nc.sync.dma_start(
                        outT[eck * 128:(eck + 1) * 128, g * GN:(g + 1) * GN],
                        ot[:])

            gtag = [0]

            def gate_eck_bf16(g, eck, cnnT, last=False):
                # gate: gateT[e, r] = sigmoid(sum_d gw[d, e] * cnnT[d, r] + gb[e])
                t = "ABCD"[gtag[0] % 4]
                gtag[0] += 1
                ps2 = gpsum.tile([128, GN], mybir.dt.float32, tag=f"gps{t}",
                                 name=f"ps2{t}")
                for dck in range(NCH):
                    nc.tensor.matmul(
                        ps2[:],
                        gwr_sb[eck][:, dck * 128:(dck + 1) * 128],
                        cnnT[dck][:],
                        start=(dck == 0),
                        stop=(dck == NCH - 1),
                    )
                gt = work.tile([128, GN], mybir.dt.bfloat16, tag="gate")
                ot = work.tile([128, GN], mybir.dt.bfloat16, tag="out")
                # the last tile's sigmoid->mul->DMA chain is the kernel
                # tail: run it in half-tiles so the stages overlap
                chunks = ((0, GN // 2), (GN // 2, GN)) if last else ((0, GN),)
                for lo, hi in chunks:
                    nc.scalar.activation(gt[:, lo:hi], ps2[:, lo:hi],
                                         AF.Sigmoid,
                                         bias=gb_sb[:, eck:eck + 1])
                    nc.vector.tensor_mul(ot[:, lo:hi], cnnT[eck][:, lo:hi],
                                         gt[:, lo:hi])
                    nc.sync.dma_start(
                        outT[eck * 128:(eck + 1) * 128,
                             g * GN + lo:g * GN + hi], ot[:, lo:hi]
                    )

            cnn_tiles = [[None] * NCH for _ in range(NG)]
            # pair A convs (fp8-gated groups)
            for dck in range(NCH):
                for g in range(NF8):
                    cnn_tiles[g][dck] = conv_tile(g, dck, make_fp8=True)
            # pair B convs with pair A's fp8 gates interleaved beneath them
            for dck in range(NCH):
                for g in range(NF8, NG):
                    cnn_tiles[g][dck] = conv_tile(g, dck, make_fp8=False)
                gts = [work.tile([128, GN], mybir.dt.bfloat16, tag="gate",
                                 name=f"gt8{g}") for g in range(NF8)]
                gate_eck_fp8(dck, cnn_tiles, gts)
            # bf16 gates form the tail, the two groups' ecks interleaved so
            # the 4 psum slots keep a deep matmul/epilogue pipeline
            for eck in range(NCH):
                for g in range(NF8, NG):
                    gate_eck_bf16(g, eck, cnn_tiles[g],
                                  last=(eck == NCH - 1 and g == NG - 1))
    nc.compile()
    return nc


def _prep_core_input(x_shard, cw_host, gw_host, gw8_host, cb_host, gb_host,
                     gb64_host):
    # x_shard: [BC, T, D] -> padded transposed [D, NG, PW, GWIN]
    xs = x_shard.reshape(NG, GWIN, W, D)
    xp = np.zeros((D, NG, PW, GWIN), np.float32)
    xp[:, :, 1:1 + W, :] = xs.transpose(3, 0, 2, 1)
    # winograd input transform, pairs p: x_i = xp[2p + i]
    x0 = xp[:, :, 0:2 * NP:2]
    x1 = xp[:, :, 1:1 + 2 * NP:2]
    x2 = xp[:, :, 2:2 + 2 * NP:2]
    x3 = xp[:, :, 3:3 + 2 * NP:2]
    xt = np.stack([x0 - x2, x1 + x2, x2 - x1, x1 - x3], axis=2)  # [D,NG,4,NP,GWIN]
    # -> [g, cc, (ck, j, p, w)] so each group is one contiguous DMA
    xt = xt.reshape(NCH, 128, NG, 4, NP, GWIN).transpose(2, 1, 0, 3, 4, 5)
    xt_host = np.ascontiguousarray(xt).astype(BF16).reshape(NG, 128, NCH * 4 * GM)
    return {"xt": xt_host, "cwr": cw_host, "gwr": gw_host, "gw8": gw8_host,
            "cb": cb_host, "gb": gb_host, "gb64": gb64_host}


def _prep_in_maps(x, conv_w, conv_b, gate_w, gate_b):
    # winograd weight transform + lhsT blocks:
    # cwr[dck][cc, (j*NCH+ck)*128 + dd] = Wt_j[dck*128+dd, ck*128+cc]
    W0, W1, W2 = conv_w[:, :, 0], conv_w[:, :, 1], conv_w[:, :, 2]
    wt = np.stack([W0, (W0 + W1 + W2) * 0.5, (W0 - W1 + W2) * 0.5, W2])  # [4,d,c]
    wt = wt.reshape(4, NCH, 128, NCH, 128)  # [j, dck, dd, ck, cc]
    cw_host = np.ascontiguousarray(wt.transpose(1, 4, 0, 3, 2)).reshape(
        NCH, 128, 4 * NCH * 128).astype(BF16)
    # gate lhsT blocks: gwr[eck][dd, dck*128 + ee] = gate_w[eck*128+ee, dck*128+dd]
    gwt = gate_w.T.reshape(NCH, 128, NCH, 128)  # [dck, dd, eck, ee]
    gw_host = np.ascontiguousarray(gwt.transpose(2, 1, 0, 3)).reshape(
        NCH, 128, NCH * 128).astype(BF16)
    # fp8 DoubleRow gate blocks: [dd, (m, s, i, ee)]
    g8 = gate_w.reshape(NE64, 64, 4, 2, 128)  # [m, ee, s, i, dd]
    gw8_host = np.ascontiguousarray(g8.transpose(4, 0, 2, 3, 1)).reshape(
        128, NE64 * 4 * 128).astype(F8)
    cb_host = np.ascontiguousarray(conv_b.reshape(NCH, 128).T).astype(np.float32)
    gb_host = np.ascontiguousarray(gate_b.reshape(NCH, 128).T).astype(np.float32)
    gb64_host = np.ascontiguousarray(gate_b.reshape(NE64, 64).T).astype(np.float32)
    return [
        _prep_core_input(x[BC * i:BC * (i + 1)], cw_host, gw_host, gw8_host,
                         cb_host, gb_host, gb64_host)
        for i in range(NCORES)
    ]


def _unshard_core(o):
    # o: [D, RC] with columns ordered (group, t, win) -> [BC, T, D]
    return (np.asarray(o).astype(np.float32)
            .reshape(D, NG, W, GWIN).transpose(1, 3, 2, 0)
            .reshape(NWIN, W, D).reshape(BC, T, D))


_NC_CACHE = None


def kernel(x, conv_w, conv_b, gate_w, gate_b):
    global _NC_CACHE
    x = np.asarray(x, np.float32)
    conv_w = np.asarray(conv_w, np.float32)
    conv_b = np.asarray(conv_b, np.float32)
    gate_w = np.asarray(gate_w, np.float32)
    gate_b = np.asarray(gate_b, np.float32)

    in_maps = _prep_in_maps(x, conv_w, conv_b, gate_w, gate_b)
    if _NC_CACHE is None:
        _NC_CACHE = _build()
    res = run_bass_kernel_spmd(_NC_CACHE, in_maps, core_ids=list(range(NCORES))).results

    out = np.empty((B, T, D), np.float32)
    for i in range(NCORES):
        out[BC * i:BC * (i + 1)] = _unshard_core(res[i]["outT"])
    return out
